# revision 35
# baseline (speedup 1.0000x reference)
"""Trainium2 Bass kernel for nn_ArgmaxPositions (argmax-position relevance scatter).

Reference computation (per (i,j,c) of a [39,39,64] grid):
  k* = argmax_{k in 256} patch(i,j)[k] * w[k,c]   (k = (px,py,pc) = px*32+py*4+pc)
  out[4i+px*, 4j+py*, pc*] += rel[i,j,c]
Output: [1,160,160,4] float32.

Distribution (8 NeuronCores, SPMD):
  - Shard Cout=64 -> 8 channels per core. Each core computes argmax+scatter for
    its channels over the full 39x39 grid into a private relevance map; a
    ReduceScatter(add) sums the maps and leaves each core a 20-gx-row slice.
  - Per core: 13 tiles of (3 i-rows x 39 j) = 117 partitions.
    DVE: prod = patch*w (broadcast over c) -> reduce_max over k -> is_equal
    (one-hot, bf16, written transposed [117,256,8]); Pool: one-hot *= rel;
    DVE: reduce_add over c -> P[117,256].  The Pool stage is hidden by
    double-buffering the one-hot and running DVE's reduce one tile behind;
    patches are triple-buffered and prefetched two tiles ahead.
  - col2im: with stride 4 / filter 8, parity groups (i%2,j%2) tile the output
    plane disjointly -> per-tile strided DMAs scatter P straight from SBUF into
    4 DRAM canvases (collision-free), overlapped with compute; canvases are
    summed with vector adds and ReduceScatter'ed at the end.
"""

import numpy as np

H_IN, W_IN, C_IN = 160, 160, 4
H_OUT, W_OUT, C_OUT = 39, 39, 64
F, S = 8, 4
N_CORES = 8
C_SH = C_OUT // N_CORES          # 8 output channels per core
K = F * F * C_IN                 # 256 patch positions
TILE_I = 3
N_TILES = H_OUT // TILE_I        # 13
NP = TILE_I * W_OUT              # 117 partitions per tile
GX_SH = H_IN // N_CORES          # 20 output rows per core
OUT_FLAT = H_IN * W_IN * C_IN    # 102400
RS_SH = OUT_FLAT // N_CORES      # 12800
FILLS_PER_TILE = 2 * TILE_I      # (b parity) x (i rows)


def _build_nc(with_tail=True, with_compute=True):
    from contextlib import ExitStack

    from concourse import bass
    import concourse.mybir as mybir

    f32 = mybir.dt.float32
    bf16 = mybir.dt.bfloat16
    AP = bass.AP
    Alu = mybir.AluOpType
    Axis = mybir.AxisListType

    nc = bass.Bass(target_bir_lowering=False, debug=True)

    patches_ext = nc.declare_dram_parameter(
        "patches", [N_TILES, NP, K], f32, isOutput=False
    )
    w_ext = nc.declare_dram_parameter("w", [C_SH, K], f32, isOutput=False)
    rel_ext = nc.declare_dram_parameter("rel", [NP, N_TILES, C_SH], bf16, isOutput=False)
    out_ext = nc.declare_dram_parameter("out", [GX_SH, W_IN, C_IN], f32, isOutput=True)

    canv = nc.dram_tensor("canv", [4, H_IN, W_IN, C_IN], bf16)
    ar_in = nc.dram_tensor("ar_in", [OUT_FLAT], f32)
    rs_out = nc.dram_tensor("rs_out", [RS_SH], f32)

    # DRAM element strides
    xs_r, xs_c = W_IN * C_IN, C_IN          # x[row, col, pc]

    with ExitStack() as ctx:
        block = ctx.enter_context(nc.Block())
        sem = lambda name: ctx.enter_context(nc.semaphore(name))
        zw_sem = sem("zw_sem")
        patch_semA = sem("patch_semA")
        patch_semB = sem("patch_semB")
        patch_semC = sem("patch_semC")
        fill_sem0 = sem("fill_sem0")
        fill_sem1 = sem("fill_sem1")
        zc_sem = sem("zc_sem")        # canvas zero DMAs
        rb_sem = sem("rb_sem")
        zwb_sem = sem("zwb_sem")   # Pool w half
        fsc_sem = sem("fsc_sem")   # scalar-issued tile-12 fills
        ar_sem = sem("ar_sem")
        z_sem = sem("z_sem")          # zero-tile memset done
        ve_sem = sem("ve_sem")        # DVE eq(t) milestones
        vr_sem = sem("vr_sem")        # DVE reduce(t) milestones
        vt_sem = sem("vt_sem")        # DVE intra-tile chain (mult/max/adds)
        p_sem = sem("p_sem")          # Pool mult milestones
        cc_sem = sem("cc_sem")
        va_sem = sem("va_sem")        # final acc sum done

        rbb_sem0 = sem("rbb_sem0")    # band readback DMAs (ping/pong)
        rbb_sem1 = sem("rbb_sem1")
        pb_sem = sem("pb_sem")        # Pool intra-band add chain
        pba_sem = sem("pba_sem")      # Pool band-acc done milestones
        arb_sem0 = sem("arb_sem0")    # ar_in band DMAs (ping/pong)
        arb_sem1 = sem("arb_sem1")
        vo_sem = sem("vo_sem")        # out cast done
        vrb_sem = sem("vrb_sem")      # last-tile reduce, upper k half
        zwc_sem = sem("zwc_sem")      # scalar-fetched upper w quarter
        pe_sem = sem("pe_sem")        # Pool eq-half milestones

        sb = lambda *a: ctx.enter_context(nc.sbuf_tensor(*a))
        w_rep = sb("w_rep", [NP, C_SH, K], f32)
        patch_sb = sb("patch_sb", [NP, 3, K], f32)
        prod = sb("prod", [NP, 2, C_SH, K], f32)
        mvals = sb("mvals", [NP, C_SH], f32)
        onehot = sb("onehot", [NP, 2, K, C_SH], bf16)
        Pbuf = sb("Pbuf", [NP, 2, K], bf16)
        rel_bf = sb("rel_bf", [NP, N_TILES, C_SH], bf16)
        zero_t = sb("zero_t", [128, 800], bf16)
        bigrb = sb("bigrb", [128, 4, 800], bf16)
        acc_bf = sb("acc_bf", [128, 800], f32)
        rs_sb = sb("rs_sb", [128, 100], bf16)
        out_sb = sb("out_sb", [128, 100], f32)

        patch_sems = [patch_semA, patch_semB, patch_semC]
        fill_sems = [fill_sem0, fill_sem1]

        def n_fill(t):  # same-parity fill groups through tile t
            return t // 2 + 1

        # ---------------- sync engine: all DMA traffic ----------------
        # scalar engine: separate DMA queue for the big w_rep broadcast, so
        # patch DMAs (sync queue) aren't stuck behind 936KB
        @block.scalar
        def _(scalar: bass.BassScalarEngine):
            # c<2 (DVE's prod half) first, then rel; Pool fetches c>=2
            scalar.dma_start(
                out=w_rep[:, 0:2, :],
                in_=AP(w_ext, 0, [[0, NP], [K, 2], [1, K]]),
            ).then_inc(zw_sem, 16)
            scalar.dma_start(
                out=rel_bf[:, :, :],
                in_=rel_ext[:, :, :],
            ).then_inc(rb_sem, 16)

            if with_tail:
                # low-row (gx<140) readback of canvases 2:4 on this queue;
                # rows >=144 are only touched by tile 12, rows<140 final
                # after fills(11)/fills(10)
                if with_compute:
                    scalar.wait_ge(
                        fill_sems[1], 16 * FILLS_PER_TILE * n_fill(N_TILES - 2)
                    )
                    scalar.wait_ge(
                        fill_sems[0], 16 * FILLS_PER_TILE * n_fill(N_TILES - 3)
                    )
                else:
                    scalar.wait_ge(zc_sem, 16 * 4)
                scalar.dma_start(
                    out=bigrb[0:112, 2:4, :],
                    in_=AP(canv, 2 * OUT_FLAT, [[800, 112], [OUT_FLAT, 2], [1, 800]]),
                ).then_inc(rbb_sem1, 16)
                if with_compute:
                    # tile 12's il=2 fills on this queue, parallel to sync's
                    t12 = N_TILES - 1
                    i12 = TILE_I * t12 + 2
                    a12 = i12 % 2
                    scalar.wait_ge(vr_sem, N_TILES)
                    for b in range(2):
                        nj = (W_OUT - b + 1) // 2
                        p0 = 2 * W_OUT + (0 if b == 0 else (W_OUT + 1) // 2)
                        scalar.dma_start(
                            out=AP(
                                canv,
                                (2 * a12 + b) * OUT_FLAT
                                + 4 * i12 * xs_r
                                + 4 * b * xs_c,
                                [[8 * xs_c, nj], [xs_r, F], [1, F * C_IN]],
                            ),
                            in_=Pbuf[p0 : p0 + nj, t12 % 2, :],
                        ).then_inc(fsc_sem, 16)

        @block.sync
        def _(sync: bass.BassEngine):

            def issue_patch(t):
                # patches are pre-gathered (im2col) host-side: one contiguous DMA
                sync.dma_start(
                    out=patch_sb[:, t % 3, :],
                    in_=AP(patches_ext, t * NP * K, [[K, NP], [1, K]]),
                ).then_inc(patch_sems[t % 3], 16)

            if with_compute:
                issue_patch(0)
                issue_patch(1)
                issue_patch(2)

            if with_tail:
                sync.wait_ge(z_sem, 1)
                for g in range(4):
                    sync.dma_start(
                        out=AP(canv, g * OUT_FLAT, [[800, 128], [1, 800]]),
                        in_=zero_t[:, :],
                    ).then_inc(zc_sem, 16)

            def issue_fills(t, ils=range(TILE_I), px_half=None):
                # scatter Pbuf[:, t%2] (tile t's 3 i-rows) into parity canvases.
                # i = 3t+il; a = i%2; canvas row gx = 4i+px; cols gy = 4j+py.
                # px_half: None = all 8 px rows; 0/1 = lower/upper 4 (k halves)
                if px_half is None:
                    pxo, npx, ko = 0, F, slice(None)
                elif px_half == 0:
                    pxo, npx, ko = 0, F // 2, slice(0, K // 2)
                else:
                    pxo, npx, ko = F // 2, F // 2, slice(K // 2, K)
                for il in ils:
                    i = TILE_I * t + il
                    a = i % 2
                    for b in range(2):
                        nj = (W_OUT - b + 1) // 2
                        p0 = il * W_OUT + (0 if b == 0 else (W_OUT + 1) // 2)
                        g = 2 * a + b
                        sync.dma_start(
                            out=AP(
                                canv,
                                g * OUT_FLAT + (4 * i + pxo) * xs_r + 4 * b * xs_c,
                                [[8 * xs_c, nj], [xs_r, npx], [1, F * C_IN]],
                            ),
                            in_=Pbuf[p0 : p0 + nj, t % 2, ko],
                        ).then_inc(fill_sems[t % 2], 16)

            if with_compute:
                for t in range(N_TILES):
                    # prefetch patch(t+3): overwrites buf t%3, last read by
                    # mult(t) (vt hits 2t+1 when mult(t) completes)
                    if t + 3 < N_TILES:
                        sync.wait_ge(vt_sem, 2 * t + 1)
                        sync.wait_ge(pe_sem, t)
                        issue_patch(t + 3)
                    sync.wait_ge(vr_sem, t + 1)
                    if with_tail:
                        if t == 0:
                            sync.wait_ge(zc_sem, 16 * 4)
                        # last tile: il=2 fills go out on the scalar queue
                        issue_fills(t, ils=(0, 1) if t == N_TILES - 1 else range(TILE_I))
                        if t == N_TILES - 1:
                            # low-row readback of canvases 0:2 (final after
                            # fills(11)/fills(10); tile 12 only writes gx>=144,
                            # disjoint from rows <140 read here)
                            sync.dma_start(
                                out=bigrb[0:112, 0:2, :],
                                in_=AP(canv, 0, [[800, 112], [OUT_FLAT, 2], [1, 800]]),
                            ).then_inc(rbb_sem0, 16)

            if with_tail:
                if with_compute:
                    sync.wait_ge(
                        fill_sems[0],
                        16 * (FILLS_PER_TILE * n_fill(N_TILES - 3) + 4),
                    )
                    sync.wait_ge(fill_sems[1], 16 * FILLS_PER_TILE * n_fill(N_TILES - 2))
                    sync.wait_ge(fsc_sem, 16 * 2)
                else:
                    sync.wait_ge(zc_sem, 16 * 4)
                # high rows (gx>=140): all 4 canvases, small
                sync.dma_start(
                    out=bigrb[112:128, :, :],
                    in_=AP(canv, 112 * 800, [[800, 16], [OUT_FLAT, 4], [1, 800]]),
                ).then_inc(rbb_sem0, 16)

                # after DVE summed + cast bf16: push to ar_in
                sync.wait_ge(va_sem, 1)
                sync.dma_start(
                    out=AP(ar_in, 0, [[800, 128], [1, 800]]),
                    in_=acc_bf[:, :],
                ).then_inc(ar_sem, 16)

                sync.wait_ge(cc_sem, 1)
                sync.dma_start(
                    out=AP(out_ext, 0, [[100, 128], [1, 100]]),
                    in_=AP(rs_out, 0, [[100, 128], [1, 100]]),
                ).then_inc(ar_sem, 16)
                sync.wait_ge(ar_sem, 32)

        # ---------------- DVE: main compute ----------------
        @block.vector
        def _(vector: bass.BassVectorEngine):
            vector.memset(zero_t[:, :], 0.0).then_inc(z_sem, 1)

            if with_compute:
                vector.wait_ge(zw_sem, 16)
                vector.wait_ge(zwb_sem, 16)  # tile 0 uses the full w

                CL2 = 2  # prod split: DVE computes c<CL2, Pool computes c>=CL2
                for t in range(N_TILES + 1):
                    if t < N_TILES:
                        cl = C_SH if t == 0 else CL2  # tile 0 fully on DVE
                        vector.wait_ge(patch_sems[t % 3], 16 * (t // 3 + 1))
                        if t >= 2:
                            # prod[t%2] WAR: eq(t-2) must be done reading it
                            vector.wait_ge(ve_sem, t - 1)
                        vector.tensor_tensor(
                            out=prod[:, t % 2, :cl, :],
                            in0=patch_sb[:, t % 3, :]
                            .unsqueeze(1)
                            .to_broadcast([NP, cl, K]),
                            in1=w_rep[:, :cl, :],
                            op=Alu.mult,
                        ).then_inc(vt_sem, 1)
                        vector.wait_ge(vt_sem, 2 * t + 1)
                        if t >= 1:
                            # Pool's prod half must be in before the max
                            vector.wait_ge(pe_sem, t)
                        vector.tensor_reduce(
                            out=mvals[:, :],
                            in_=prod[:, t % 2, :, :],
                            axis=Axis.X,
                            op=Alu.max,
                        ).then_inc(vt_sem, 1)
                        vector.wait_ge(vt_sem, 2 * t + 2)
                        if t >= 2:
                            # onehot[t%2] overwrite: Pool mult(t-2) done
                            vector.wait_ge(p_sem, t - 1)
                        vector.tensor_tensor(
                            out=onehot[:, t % 2, :, :].transpose([0, 2, 1]),
                            in0=prod[:, t % 2, :, :],
                            in1=mvals[:, :].unsqueeze(2).to_broadcast([NP, C_SH, K]),
                            op=Alu.is_equal,
                        ).then_inc(ve_sem, 1)
                    if t >= 1:
                        tr = t - 1
                        vector.wait_ge(p_sem, tr + 1)
                        if with_tail and tr >= 2:
                            # Pbuf[tr%2] reuse: fills(tr-2) must have drained it
                            vector.wait_ge(
                                fill_sems[tr % 2], 16 * FILLS_PER_TILE * n_fill(tr - 2)
                            )
                        with nc.allow_low_precision(
                            "bf16 relevance sums stay well inside the 2e-2 gate"
                        ):
                            vector.tensor_reduce(
                                out=Pbuf[:, tr % 2, :],
                                in_=onehot[:, tr % 2, :, :],
                                axis=Axis.X,
                                op=Alu.add,
                            ).then_inc(vr_sem, 1)

            if with_tail:
                nvt = 2 * N_TILES if with_compute else 0
                vector.wait_ge(rbb_sem0, 32)
                vector.tensor_tensor(
                    out=bigrb[:, 0, :], in0=bigrb[:, 0, :], in1=bigrb[:, 1, :],
                    op=Alu.add,
                ).then_inc(vt_sem, 1)
                vector.wait_ge(rbb_sem1, 16)
                vector.tensor_tensor(
                    out=bigrb[:, 2, :], in0=bigrb[:, 2, :], in1=bigrb[:, 3, :],
                    op=Alu.add,
                ).then_inc(vt_sem, 1)
                vector.wait_ge(vt_sem, nvt + 2)
                # final add writes f32 directly (dtype converts on write)
                vector.tensor_tensor(
                    out=acc_bf[:, :], in0=bigrb[:, 0, :], in1=bigrb[:, 2, :],
                    op=Alu.add,
                ).then_inc(va_sem, 1)

        # ---------------- Pool: rel multiply + collective ----------------
        @block.gpsimd
        def _(gpsimd: bass.BassGpSimd):
            if with_compute:
                CL2 = 2
                # fetch own w half on the Pool DMA queue, parallel to scalar's
                gpsimd.dma_start(
                    out=w_rep[:, 2:, :],
                    in_=AP(w_ext, 2 * K, [[0, NP], [K, C_SH - 2], [1, K]]),
                ).then_inc(zwb_sem, 16)
                gpsimd.wait_ge(zwb_sem, 16)
                gpsimd.wait_ge(rb_sem, 16)  # rel_bf loaded

                def rel_mult(t):
                    gpsimd.wait_ge(ve_sem, t + 1)
                    gpsimd.tensor_tensor(
                        out=onehot[:, t % 2, :, :],
                        in0=onehot[:, t % 2, :, :],
                        in1=rel_bf[:, t, :].unsqueeze(1).to_broadcast([NP, K, C_SH]),
                        op=Alu.mult,
                    ).then_inc(p_sem, 1)

                for t in range(1, N_TILES):
                    # upper prod half: prod[:, t%2, CL2:, :] = patch * w
                    gpsimd.wait_ge(patch_sems[t % 3], 16 * (t // 3 + 1))
                    if t >= 2:
                        # prod[t%2] WAR: eq(t-2) must be done reading it
                        gpsimd.wait_ge(ve_sem, t - 1)
                    gpsimd.tensor_tensor(
                        out=prod[:, t % 2, CL2:, :],
                        in0=patch_sb[:, t % 3, :]
                        .unsqueeze(1)
                        .to_broadcast([NP, C_SH - CL2, K]),
                        in1=w_rep[:, CL2:, :],
                        op=Alu.mult,
                    ).then_inc(pe_sem, 1)
                    rel_mult(t - 1)
                rel_mult(N_TILES - 1)

            if with_tail:
                gpsimd.wait_ge(ar_sem, 16)
                gpsimd.collective_compute(
                    "ReduceScatter",
                    Alu.add,
                    replica_groups=[list(range(N_CORES))],
                    ins=[ar_in[:]],
                    outs=[rs_out[:]],
                ).then_inc(cc_sem, 1)

    return nc


_NC = None


def _get_nc():
    global _NC
    if _NC is None:
        _NC = _build_nc()
    return _NC


LAST_RESULT = None


def kernel(inputs, layer_output, layer_weights, stride=4, filter_size=8, **_kw):
    assert int(stride) == S and int(filter_size) == F
    rel = np.asarray(inputs, dtype=np.float32)[0]          # [39,39,64]
    x = np.ascontiguousarray(np.asarray(layer_output, dtype=np.float32)[0])
    w = np.asarray(layer_weights, dtype=np.float32)        # [8,8,4,64]

    # host-side im2col in the kernel's (il*39+jp, t, k) layout, j parity-permuted
    j_order = list(range(0, W_OUT, 2)) + list(range(1, W_OUT, 2))
    idx_r = (S * np.arange(H_OUT))[:, None] + np.arange(F)[None, :]
    idx_c = (S * np.asarray(j_order))[:, None] + np.arange(F)[None, :]
    pat = x[idx_r][:, :, idx_c, :]                    # [i, px, jp, py, pc]
    pat = pat.transpose(0, 2, 1, 3, 4).reshape(H_OUT, W_OUT, K)
    patches = np.ascontiguousarray(
        pat.reshape(N_TILES, TILE_I, W_OUT, K).reshape(N_TILES, NP, K)
    )

    from concourse.bass_utils import run_bass_kernel_spmd

    nc = _get_nc()
    in_maps = []
    for r in range(N_CORES):
        cs = slice(C_SH * r, C_SH * (r + 1))
        w_t = np.ascontiguousarray(
            w[:, :, :, cs].transpose(3, 0, 1, 2).reshape(C_SH, K)
        )
        j_order = list(range(0, W_OUT, 2)) + list(range(1, W_OUT, 2))
        import ml_dtypes

        rel_r = np.ascontiguousarray(
            rel[:, j_order, :][:, :, cs]
            .reshape(N_TILES, TILE_I, W_OUT, C_SH)
            .transpose(1, 2, 0, 3)
            .reshape(NP, N_TILES, C_SH)
            .astype(ml_dtypes.bfloat16)
        )
        in_maps.append({"patches": patches, "w": w_t, "rel": rel_r})

    import os

    trace = bool(int(os.environ.get("KERNEL_TRACE", "0")))
    res = run_bass_kernel_spmd(nc, in_maps, list(range(N_CORES)), trace=trace)
    global LAST_RESULT
    LAST_RESULT = res
    slices = [np.asarray(res.results[r]["out"]) for r in range(N_CORES)]
    out = np.concatenate(slices, axis=0).reshape(1, H_IN, W_IN, C_IN)
    return out.astype(np.float32)



# revision 36
# speedup vs baseline: 1.1655x; 1.1655x over previous
"""Trainium2 Bass kernel for nn_ArgmaxPositions (argmax-position relevance scatter).

Reference computation (per (i,j,c) of a [39,39,64] grid):
  k* = argmax_{k in 256} patch(i,j)[k] * w[k,c]   (k = (px,py,pc) = px*32+py*4+pc)
  out[4i+px*, 4j+py*, pc*] += rel[i,j,c]
Output: [1,160,160,4] float32.

Distribution (8 NeuronCores, SPMD): shard Cout=64 -> 8 channels per core.
Each core computes argmax+scatter for its channels over the full 39x39 grid
into a private relevance map; ReduceScatter(add) sums the maps and leaves
each core a 20-gx-row slice.

Per-core pipeline (13 tiles of 3 i-rows x 40 j-slots = 120 partitions):
  - channels split: DVE computes prod=patch*w and the per-channel max for
    c3..c7; Pool does the same for c0..c2 (fully disjoint buffers).
  - one-hot * rel, split by channel:
      Act (c3..c7): s = Sign(-prod + mx) in {0,1}; q = Identity(s*(-rel)+rel)
        -> exactly rel at the argmax, 0 elsewhere (per-partition scale/bias APs).
      Pool (c0..c2): per-c fused STT q = (prod == mx_scalar) * rel_bcast.
  - c-reduction: bf16 add tree on DVE -> P[p, 256] bf16.
  - col2im scatter: canvases use a j-SLOT layout (row = 40 slots x 32), so
    writes from adjacent j never superimpose; only i-parity needs 2 canvases.
    3 strided DMAs per tile scatter P straight to DRAM, overlapped.
  - tail: row-aligned readback (row gx = slot*128 + p), pair-add, then one
    shifted add per slot un-slots (each gy sums exactly two (j,py) terms),
    push f32, ReduceScatter(add), copy the 20-row slice out.
"""

import numpy as np

H_IN, W_IN, C_IN = 160, 160, 4
H_OUT, W_OUT, C_OUT = 39, 39, 64
F, S = 8, 4
N_CORES = 8
C_SH = C_OUT // N_CORES          # 8 output channels per core
K = F * F * C_IN                 # 256 patch positions
TILE_I = 3
N_TILES = H_OUT // TILE_I        # 13
JS = 40                          # j-slots per row (39 real + 1 zero pad)
NP = TILE_I * JS                 # 120 partitions per tile
ROW = JS * F * C_IN              # 1280 slot-elements per canvas row
CANV = H_IN * ROW                # 204800 elements per canvas
OUT_FLAT = H_IN * W_IN * C_IN    # 102400
RS_SH = OUT_FLAT // N_CORES      # 12800
GX_SH = H_IN // N_CORES          # 20 output rows per core
WC = W_IN * C_IN                 # 640

# channel assignment: Pool computes one-hot*rel for c in [0, CP), Act the rest;
# DVE computes mult+max for channels [CP, 8), Pool for [0, CP)
CP = 8


def _build_nc(with_tail=True, with_compute=True):
    from contextlib import ExitStack

    from concourse import bass
    import concourse.mybir as mybir

    f32 = mybir.dt.float32
    bf16 = mybir.dt.bfloat16
    AP = bass.AP
    Alu = mybir.AluOpType
    ActF = mybir.ActivationFunctionType

    nc = bass.Bass(target_bir_lowering=False, debug=True)

    patches_ext = nc.declare_dram_parameter(
        "patches", [N_TILES, NP, K], f32, isOutput=False
    )
    w_ext = nc.declare_dram_parameter("w", [C_SH, K], f32, isOutput=False)
    # rel2[p, t, c, 0] = +rel, rel2[p, t, c, 1] = -rel (f32: Act scale/bias APs)
    rel_ext = nc.declare_dram_parameter(
        "rel", [NP, N_TILES, C_SH, 2], f32, isOutput=False
    )
    out_ext = nc.declare_dram_parameter("out", [GX_SH, W_IN, C_IN], f32, isOutput=True)

    canv = nc.dram_tensor("canv", [2, CANV], bf16)     # a = i%2 slot canvases
    dbg_dram = nc.dram_tensor("dbg_dram", [NP * K], bf16)
    dbg_f32 = nc.dram_tensor("dbg_f32", [NP * C_SH], f32)
    ar_in = nc.dram_tensor("ar_in", [OUT_FLAT], f32)
    rs_out = nc.dram_tensor("rs_out", [RS_SH], f32)

    NA = C_SH - CP            # channels on Act
    AI = 2 * NA               # Act instrs per tile

    with ExitStack() as ctx:
        block = ctx.enter_context(nc.Block())
        sem = lambda name: ctx.enter_context(nc.semaphore(name))
        pt_sem = sem("pt_sem")    # patch pair DMAs
        zw_sem = sem("zw_sem")    # w_rep load, Act's channels (c4..7)
        zwb_sem = sem("zwb_sem")  # w_rep load, Pool's channels (c0..3)
        rl_sem = sem("rl_sem")    # rel load
        zc_sem = sem("zc_sem")    # canvas-1 edge zeroing
        z_sem = sem("z_sem")      # zero_t memset + acc guards
        tr_sem = sem("tr_sem")    # DVE progress: max8, 3x STT (+4 per tile)
        pe_sem = sem("pe_sem")    # Pool progress: mult8 (+1 per tile)
        ak_sem = sem("ak_sem")    # Act instr progress (+AI per tile)
        dv_sem = sem("dv_sem")    # DVE tree lvl1/2/3 (+3 per tile)
        fl_sem = sem("fl_sem")    # fill DMAs (+48 per tile)
        rbs_sem = sem("rbs_sem")  # readbacks
        va_sem = sem("va_sem")    # tail adds progress
        ar_sem = sem("ar_sem")    # ar_in pushed / out written
        cc_sem = sem("cc_sem")    # collective done

        sb = lambda *a: ctx.enter_context(nc.sbuf_tensor(*a))
        w_rep = sb("w_rep", [NP, C_SH, K], f32)
        patch_sb = sb("patch_sb", [NP, 4, K], f32)
        rel2 = sb("rel2", [NP, N_TILES, C_SH, 2], f32)
        prod = sb("prod", [NP, 2, C_SH, K], f32)
        mx = sb("mx", [NP, 2, C_SH], f32)
        tmp_s = sb("tmp_s", [NP, K], bf16)          # Act Sign scratch
        q = sb("q", [NP, 2, C_SH, K], bf16)  # channel-major: all writes contiguous
        u = sb("u", [NP, 4, K], bf16)
        v = sb("v", [NP, 2, K], bf16)
        Pbuf = sb("Pbuf", [NP, 3, K], bf16)
        zero_t = sb("zero_t", [128, ROW], bf16)
        # [p, slot, canvas, half-slot h, py%4, pc]; h = j*2 + py//4
        bigrb = sb("bigrb", [128, 2, 2, 2 * JS, 4, C_IN], bf16)
        # 16-elem zero guard in front (h slot 0): h index shifted by 1
        acc2 = sb("acc2", [128, 2, 2 * JS + 1, 4, C_IN], bf16)
        accf = sb("accf", [128, 2, JS, C_IN, C_IN], f32)   # [p, slot, gy//4, gy%4, pc]
        psnap = sb("psnap", [NP, K], bf16)
        psnap2 = sb("psnap2", [NP, K], bf16)

        # ---------------- sync engine: patches + fills + tail DMAs ----------
        @block.sync
        def _(sync: bass.BassEngine):
            if with_compute:
                # patch pairs: tiles (2m, 2m+1) -> slots (2m%4, 2m%4+1)
                def load_pair(m):
                    nt = min(2, N_TILES - 2 * m)
                    sync.dma_start(
                        out=patch_sb[:, (2 * m) % 4 : (2 * m) % 4 + nt, :],
                        in_=AP(
                            patches_ext,
                            2 * m * NP * K,
                            [[K, NP], [NP * K, nt], [1, K]],
                        ),
                    ).then_inc(pt_sem, 16)

                # tile-0 patch alone first: it gates Pool's first mult and
                # the shared DMA device serializes transfers
                sync.dma_start(
                    out=patch_sb[:, 0:1, :],
                    in_=AP(patches_ext, 0, [[K, NP], [1, K]]),
                ).then_inc(pt_sem, 16)
                sync.dma_start(
                    out=w_rep[:, 4:, :],
                    in_=AP(w_ext, 4 * K, [[0, NP], [K, C_SH - 4], [1, K]]),
                ).then_inc(zwb_sem, 16)
                sync.dma_start(
                    out=patch_sb[:, 1:2, :],
                    in_=AP(patches_ext, NP * K, [[K, NP], [1, K]]),
                ).then_inc(pt_sem, 16)
                load_pair(1)
                for t in range(N_TILES):
                    # prefetch pair m = t//2 + 2 once tile 2m-3's readers done
                    if t % 2 == 0 and t // 2 + 2 <= (N_TILES - 1) // 2:
                        m = t // 2 + 2
                        sync.wait_ge(pe_sem, 2 * m - 2)
                        load_pair(m)
                    import os as _os3
                    sync.wait_ge(dv_sem, min(3 * (t + 2), 3 * N_TILES))
                    if with_tail:
                        if t == 0:
                            sync.wait_ge(zc_sem, 16)
                        for il in range(TILE_I):
                            i = TILE_I * t + il
                            a = i % 2
                            sync.dma_start(
                                out=AP(
                                    canv,
                                    a * CANV + 4 * i * ROW,
                                    [[F * C_IN, JS], [ROW, F], [1, F * C_IN]],
                                ),
                                in_=Pbuf[il * JS : (il + 1) * JS, t % 3, :],
                            ).then_inc(fl_sem, 16)
                        if t == 0 and _os3.environ.get("DBG_P0"):
                            sync.dma_start(
                                out=AP(dbg_dram, 0, [[K, NP], [1, K]]),
                                in_=Pbuf[:, 0, :],
                            ).then_inc(fl_sem, 16)
                        if t == 0 and _os3.environ.get("DBG_Q0"):
                            sync.dma_start(
                                out=AP(dbg_f32, 0, [[C_SH, NP], [1, C_SH]]),
                                in_=mx[:, 0, :],
                            ).then_inc(fl_sem, 16)
                            sync.dma_start(
                                out=AP(dbg_dram, 0, [[48, NP], [1, 48]]),
                                in_=qL[:, 0, 0:12, :],
                            ).then_inc(fl_sem, 16)
                            sync.dma_start(
                                out=AP(dbg_dram, NP * 48, [[48, NP], [1, 48]]),
                                in_=qH[:, 0, 0:12, :],
                            ).then_inc(fl_sem, 16)
                        if t == N_TILES - 3:
                            # rows 0-127 (slot 0) final after fills(10)
                            sync.wait_ge(fl_sem, 48 * (N_TILES - 2))
                            sync.dma_start(
                                out=bigrb[:, 0, :, :, :, :],
                                in_=AP(canv, 0, [[ROW, 128], [CANV, 2], [1, ROW]]),
                            ).then_inc(rbs_sem, 16)

            if with_tail:
                # push slot 0 (rows 0-127) once its unslot is done
                sync.wait_ge(va_sem, 2)
                sync.dma_start(
                    out=AP(ar_in, 0, [[WC, 128], [1, WC]]),
                    in_=accf[:, 0, :, :, :],
                ).then_inc(ar_sem, 16)
                sync.wait_ge(va_sem, 4)
                sync.dma_start(
                    out=AP(ar_in, 128 * WC, [[WC, 32], [1, WC]]),
                    in_=accf[0:32, 1, :, :, :],
                ).then_inc(ar_sem, 16)

                sync.wait_ge(cc_sem, 1)
                import os as _os
                if _os.environ.get("DBG_DUMP"):
                    sync.wait_ge(ar_sem, 32)
                else:
                    sync.dma_start(
                        out=AP(out_ext, 0, [[1, RS_SH]]),
                        in_=AP(rs_out, 0, [[1, RS_SH]]),
                    ).then_inc(ar_sem, 16)
                    sync.wait_ge(ar_sem, 48)

        # ---------------- scalar engine (Activation): loads + one-hot -------
        @block.scalar
        def _(scalar: bass.BassScalarEngine):
            # half of w first (other half on the sync queue, in parallel)
            scalar.dma_start(
                out=w_rep[:, 0:4, :],
                in_=AP(w_ext, 0, [[0, NP], [K, 4], [1, K]]),
            ).then_inc(zw_sem, 16)
            scalar.dma_start(
                out=rel2[:, :, :, :], in_=rel_ext[:, :, :, :]
            ).then_inc(rl_sem, 16)
            # preload the activation function table while DMAs stream in
            scalar.activation(
                out=tmp_s[0:1, 0:4],
                in_=tmp_s[0:1, 0:4],
                func=ActF.Sign,
                bias=0.0,
                scale=1.0,
            )
            if with_tail:
                # zero canvas-1 rows {0..3, 156..159} (i odd covers gx 4..155)
                scalar.wait_ge(z_sem, 1)
                scalar.dma_start(
                    out=AP(canv, CANV, [[156 * ROW, 2], [ROW, 4], [1, ROW]]),
                    in_=zero_t[0:8, :],
                ).then_inc(zc_sem, 16)

            if with_compute:
                scalar.wait_ge(rl_sem, 16)
                for t in range(N_TILES):
                    for ci, c in enumerate(range(CP, C_SH)):
                        if ci == 0:
                            # DVE's max lvl2 for tile t done
                            scalar.wait_ge(tr_sem, 9 * t + 1)
                        # s = Sign(-prod + mx) in {0 (argmax), +1}
                        scalar.activation(
                            out=tmp_s[:, :],
                            in_=prod[:, t % 2, c, :],
                            func=ActF.Sign,
                            bias=mx[:, t % 2, c : c + 1],
                            scale=-1.0,
                        ).then_inc(ak_sem, 1)
                        if ci == 0 and t >= 2:
                            # q[t%2] WAR: DVE lvl1(t-2) must have consumed it
                            scalar.wait_ge(dv_sem, 3 * (t - 2) + 1)
                        # q = s*(-rel) + rel  -> rel at argmax, 0 elsewhere
                        scalar.activation(
                            out=q[:, t % 2, c, :],
                            in_=tmp_s[:, :],
                            func=ActF.Identity,
                            bias=rel2[:, t, c, 0:1],
                            scale=rel2[:, t, c, 1:2],
                        ).then_inc(ak_sem, 1)

            if with_tail:
                if with_compute:
                    scalar.wait_ge(fl_sem, 48 * N_TILES)
                else:
                    scalar.wait_ge(zc_sem, 16)
                scalar.dma_start(
                    out=bigrb[0:32, 1, :, :, :, :],
                    in_=AP(canv, 128 * ROW, [[ROW, 32], [CANV, 2], [1, ROW]]),
                ).then_inc(rbs_sem, 16)

        # ---------------- DVE: max8 + STT one-hot c0-2 + add tree -----------
        @block.vector
        def _(vector: bass.BassVectorEngine):
            # canvas-zero source + acc guards: DVE idles at boot anyway
            vector.memset(zero_t[:, :], 0.0)
            vector.memset(acc2[:, :, 0:1, :, :], 0.0)
            vector.memset(acc2[:, 1, :, :, :], 0.0).then_inc(z_sem, 1)

            def tree(tr):
                # q[tr%2] complete: Act(tr) done (own STTs are program-order)
                vector.wait_ge(ak_sem, AI * min(tr + 1, N_TILES - 1))
                vector.tensor_tensor(
                    out=u[:, :, :],
                    in0=q[:, tr % 2, 0:4, :],
                    in1=q[:, tr % 2, 4:8, :],
                    op=Alu.add,
                ).then_inc(dv_sem, 1)
                vector.tensor_tensor(
                    out=v[:, :, :],
                    in0=u[:, 0:2, :],
                    in1=u[:, 2:4, :],
                    op=Alu.add,
                ).then_inc(dv_sem, 1)
                if with_tail and tr >= 3:
                    vector.wait_ge(fl_sem, 48 * (tr - 2))
                vector.tensor_tensor(
                    out=Pbuf[:, tr % 3, :],
                    in0=v[:, 0, :],
                    in1=v[:, 1, :],
                    op=Alu.add,
                ).then_inc(dv_sem, 1)

            if with_compute:
                vector.wait_ge(rl_sem, 16)
                with nc.allow_low_precision("bf16 one-hot relevance pipeline"):
                    for t in range(N_TILES):
                        import os as _os2
                        if _os2.environ.get("DBG_SERIAL") and t >= 2:
                            vector.wait_ge(fl_sem, 48 * (t - 1))
                        # Pool's mult for tile t done
                        vector.wait_ge(pe_sem, t + 1)
                        if t >= 2:
                            # mx[t%2] WAR: Act(t-2) done reading
                            vector.wait_ge(ak_sem, AI * (t - 1))
                        vector.tensor_reduce(
                            out=mx[:, t % 2, :],
                            in_=prod[:, t % 2, :, :],
                            axis=mybir.AxisListType.X,
                            op=Alu.max,
                        ).then_inc(tr_sem, 1)
                        # fused one-hot*rel (all 8 channels on the last
                        # tile so Act retires one tile earlier)
                        for c in range(C_SH if t == N_TILES - 1 else CP):
                            vector.scalar_tensor_tensor(
                                out=q[:, t % 2, c, :],
                                in0=prod[:, t % 2, c, :],
                                scalar=mx[:, t % 2, c : c + 1],
                                in1=rel2[:, t, c, 0]
                                .unsqueeze(1)
                                .to_broadcast([NP, K]),
                                op0=Alu.is_equal,
                                op1=Alu.mult,
                            ).then_inc(tr_sem, 1)
                        if t >= 1:
                            tree(t - 1)
                            import os as _osA
                            if t == 1 and _osA.environ.get("DBG_PSNAP"):
                                vector.tensor_scalar(
                                    out=psnap[:, :],
                                    in0=Pbuf[:, 0, :],
                                    scalar1=1.0,
                                    scalar2=None,
                                    op0=Alu.mult,
                                )
                    tree(N_TILES - 1)
                    import os as _osB
                    if _osB.environ.get("DBG_PSNAP"):
                        vector.tensor_scalar(
                            out=psnap2[:, :],
                            in0=Pbuf[:, (N_TILES - 1) % 3, :],
                            scalar1=1.0,
                            scalar2=None,
                            op0=Alu.mult,
                        )

            if with_tail:
                with nc.allow_low_precision("bf16 canvas sums"):
                    vector.wait_ge(rbs_sem, 16)
                    vector.wait_ge(z_sem, 1)
                    vector.tensor_tensor(
                        out=acc2[:, 0, 1:, :, :],
                        in0=bigrb[:, 0, 0, :, :, :],
                        in1=bigrb[:, 0, 1, :, :, :],
                        op=Alu.add,
                    ).then_inc(va_sem, 1)
                    # un-slot: out[gx, gy, pc] = slot[j(gy), py<4] + slot[j-1, py+4]
                    vector.tensor_tensor(
                        out=accf[:, 0, :, :, :],
                        in0=acc2[:, 0, 1 : 2 * JS + 1 : 2, :, :],
                        in1=acc2[:, 0, 0 : 2 * JS : 2, :, :],
                        op=Alu.add,
                    ).then_inc(va_sem, 1)
                    vector.wait_ge(rbs_sem, 32)
                    vector.tensor_tensor(
                        out=acc2[0:32, 1, 1:, :, :],
                        in0=bigrb[0:32, 1, 0, :, :, :],
                        in1=bigrb[0:32, 1, 1, :, :, :],
                        op=Alu.add,
                    ).then_inc(va_sem, 1)
                    vector.tensor_tensor(
                        out=accf[0:32, 1, :, :, :],
                        in0=acc2[0:32, 1, 1 : 2 * JS + 1 : 2, :, :],
                        in1=acc2[0:32, 1, 0 : 2 * JS : 2, :, :],
                        op=Alu.add,
                    ).then_inc(va_sem, 1)

        # ---------------- Pool: one-hot*rel (c<CP) + lvl1 + collective ------

        @block.gpsimd
        def _(gpsimd: bass.BassGpSimd):
            if with_compute:
                gpsimd.wait_ge(zw_sem, 16)
                gpsimd.wait_ge(zwb_sem, 16)
                with nc.allow_low_precision("bf16 one-hot relevance pipeline"):
                    for t in range(N_TILES):
                        gpsimd.wait_ge(pt_sem, 16 * (t + 1) if t < 2 else 16 * (t // 2 + 2))
                        if t >= 2:
                            # prod[t%2] WAR: Act(t-2) + DVE(t-2) done reading
                            gpsimd.wait_ge(ak_sem, AI * (t - 1))
                            gpsimd.wait_ge(tr_sem, 9 * (t - 1))
                        gpsimd.tensor_tensor(
                            out=prod[:, t % 2, :, :],
                            in0=patch_sb[:, t % 4, :]
                            .unsqueeze(1)
                            .to_broadcast([NP, C_SH, K]),
                            in1=w_rep[:, :, :],
                            op=Alu.mult,
                        ).then_inc(pe_sem, 1)

            if with_tail:
                import os as _os
                if _os.environ.get("DBG_DUMP"):
                    gpsimd.wait_ge(fl_sem, 48 * N_TILES)
                    _r0 = int(_os.environ.get("DBG_ROW", "0"))
                    _cv = int(_os.environ.get("DBG_CANV", "0"))
                    gpsimd.dma_start(
                        out=AP(out_ext, 0, [[1, 12800]]),
                        in_=AP(canv, _cv * CANV + _r0 * ROW, [[1, 12800]]),
                    ).then_inc(ar_sem, 16)
                gpsimd.wait_ge(ar_sem, 32)
                gpsimd.collective_compute(
                    "ReduceScatter",
                    mybir.AluOpType.add,
                    replica_groups=[list(range(N_CORES))],
                    ins=[ar_in[:]],
                    outs=[rs_out[:]],
                ).then_inc(cc_sem, 1)

    return nc


_NC = None


def _get_nc():
    global _NC
    if _NC is None:
        _NC = _build_nc()
    return _NC


LAST_RESULT = None


def kernel(inputs, layer_output, layer_weights, stride=4, filter_size=8, **_kw):
    assert int(stride) == S and int(filter_size) == F
    rel = np.asarray(inputs, dtype=np.float32)[0]          # [39,39,64]
    x = np.ascontiguousarray(np.asarray(layer_output, dtype=np.float32)[0])
    w = np.asarray(layer_weights, dtype=np.float32)        # [8,8,4,64]

    import ml_dtypes

    # host-side im2col in (t, il*40+j, k) layout, natural j order, j=39 padded
    idx_r = (S * np.arange(H_OUT))[:, None] + np.arange(F)[None, :]
    idx_c = (S * np.arange(W_OUT))[:, None] + np.arange(F)[None, :]
    pat = x[idx_r][:, :, idx_c, :]                    # [i, px, j, py, pc]
    pat = pat.transpose(0, 2, 1, 3, 4).reshape(H_OUT, W_OUT, K)
    pat40 = np.concatenate([pat, pat[:, -1:, :]], axis=1)   # pad j=39 (finite)
    patches = np.ascontiguousarray(pat40.reshape(N_TILES, NP, K))

    from concourse.bass_utils import run_bass_kernel_spmd

    nc = _get_nc()
    in_maps = []
    for r in range(N_CORES):
        cs = slice(C_SH * r, C_SH * (r + 1))
        w_t = np.ascontiguousarray(
            w[:, :, :, cs].transpose(3, 0, 1, 2).reshape(C_SH, K)
        )
        rel_r = rel[:, :, cs]                              # [39, 39, 8]
        rel_p = np.zeros((H_OUT, JS, C_SH, 2), dtype=np.float32)
        rel_p[:, :W_OUT, :, 0] = rel_r
        rel_p[:, :W_OUT, :, 1] = -rel_r
        rel_p = np.ascontiguousarray(
            rel_p.reshape(N_TILES, NP, C_SH, 2).transpose(1, 0, 2, 3)
        )
        in_maps.append({"patches": patches, "w": w_t, "rel": rel_p})

    import os

    trace = bool(int(os.environ.get("KERNEL_TRACE", "0")))
    res = run_bass_kernel_spmd(nc, in_maps, list(range(N_CORES)), trace=trace)
    global LAST_RESULT
    LAST_RESULT = res
    slices = [np.asarray(res.results[r]["out"]) for r in range(N_CORES)]
    out = np.concatenate(slices, axis=0).reshape(1, H_IN, W_IN, C_IN)
    return out.astype(np.float32)


# revision 37
# speedup vs baseline: 1.2997x; 1.1152x over previous
"""Trainium2 Bass kernel for nn_ArgmaxPositions (argmax-position relevance scatter).

Reference computation (per (i,j,c) of a [39,39,64] grid):
  k* = argmax_{k in 256} patch(i,j)[k] * w[k,c]   (k = (px,py,pc) = px*32+py*4+pc)
  out[4i+px*, 4j+py*, pc*] += rel[i,j,c]
Output: [1,160,160,4] float32.

Distribution (8 NeuronCores, SPMD): shard Cout=64 -> 8 channels per core.
Each core computes argmax+scatter for its channels over the full 39x39 grid
into a private relevance map; ReduceScatter(add) sums the maps and leaves
each core a 20-gx-row slice.

Per-core pipeline (13 tiles of 3 i-rows x 40 j-slots = 120 partitions):
  - channels split: DVE computes prod=patch*w and the per-channel max for
    c3..c7; Pool does the same for c0..c2 (fully disjoint buffers).
  - one-hot * rel, split by channel:
      Act (c3..c7): s = Sign(-prod + mx) in {0,1}; q = Identity(s*(-rel)+rel)
        -> exactly rel at the argmax, 0 elsewhere (per-partition scale/bias APs).
      Pool (c0..c2): per-c fused STT q = (prod == mx_scalar) * rel_bcast.
  - c-reduction: bf16 add tree on DVE -> P[p, 256] bf16.
  - col2im scatter: canvases use a j-SLOT layout (row = 40 slots x 32), so
    writes from adjacent j never superimpose; only i-parity needs 2 canvases.
    3 strided DMAs per tile scatter P straight to DRAM, overlapped.
  - tail: row-aligned readback (row gx = slot*128 + p), pair-add, then one
    shifted add per slot un-slots (each gy sums exactly two (j,py) terms),
    push f32, ReduceScatter(add), copy the 20-row slice out.
"""

import numpy as np

H_IN, W_IN, C_IN = 160, 160, 4
H_OUT, W_OUT, C_OUT = 39, 39, 64
F, S = 8, 4
N_CORES = 8
C_SH = C_OUT // N_CORES          # 8 output channels per core
K = F * F * C_IN                 # 256 patch positions
TILE_I = 3
N_TILES = H_OUT // TILE_I        # 13
JS = 40                          # j-slots per row (39 real + 1 zero pad)
NP = TILE_I * JS                 # 120 partitions per tile
ROW = JS * F * C_IN              # 1280 slot-elements per canvas row
CANV = H_IN * ROW                # 204800 elements per canvas
OUT_FLAT = H_IN * W_IN * C_IN    # 102400
RS_SH = OUT_FLAT // N_CORES      # 12800
GX_SH = H_IN // N_CORES          # 20 output rows per core
WC = W_IN * C_IN                 # 640

# channel assignment: Pool computes one-hot*rel for c in [0, CP), Act the rest;
# DVE computes mult+max for channels [CP, 8), Pool for [0, CP)
CP = 4


def _build_nc(with_tail=True, with_compute=True):
    from contextlib import ExitStack

    from concourse import bass
    import concourse.mybir as mybir

    f32 = mybir.dt.float32
    bf16 = mybir.dt.bfloat16
    AP = bass.AP
    Alu = mybir.AluOpType
    ActF = mybir.ActivationFunctionType

    nc = bass.Bass(target_bir_lowering=False, debug=True)

    patches_ext = nc.declare_dram_parameter(
        "patches", [N_TILES, NP, K], f32, isOutput=False
    )
    w_ext = nc.declare_dram_parameter("w", [C_SH, K], f32, isOutput=False)
    # rel2[p, t, c, 0] = +rel, rel2[p, t, c, 1] = -rel (f32: Act scale/bias APs)
    rel_ext = nc.declare_dram_parameter(
        "rel", [NP, N_TILES, C_SH, 2], f32, isOutput=False
    )
    out_ext = nc.declare_dram_parameter("out", [GX_SH, W_IN, C_IN], f32, isOutput=True)

    canv = nc.dram_tensor("canv", [2, CANV], bf16)     # a = i%2 slot canvases
    dbg_dram = nc.dram_tensor("dbg_dram", [NP * K], bf16)
    dbg_f32 = nc.dram_tensor("dbg_f32", [NP * C_SH], f32)
    ar_in = nc.dram_tensor("ar_in", [OUT_FLAT], f32)
    rs_out = nc.dram_tensor("rs_out", [RS_SH], f32)

    NA = C_SH - CP            # channels on Act
    AI = 2 * NA               # Act instrs per tile

    with ExitStack() as ctx:
        block = ctx.enter_context(nc.Block())
        sem = lambda name: ctx.enter_context(nc.semaphore(name))
        pt_sem = sem("pt_sem")    # patch pair DMAs
        zw_sem = sem("zw_sem")    # w_rep load, Act's channels (c4..7)
        zwb_sem = sem("zwb_sem")  # w_rep load, Pool's channels (c0..3)
        rl_sem = sem("rl_sem")    # rel load
        zc_sem = sem("zc_sem")    # canvas-1 edge zeroing
        z_sem = sem("z_sem")      # zero_t memset + acc guards
        tr_sem = sem("tr_sem")    # DVE progress: max8, 3x STT (+4 per tile)
        pe_sem = sem("pe_sem")    # Pool progress: mult8 (+1 per tile)
        ak_sem = sem("ak_sem")    # Act instr progress (+AI per tile)
        dv_sem = sem("dv_sem")    # DVE tree lvl1/2/3 (+3 per tile)
        fl_sem = sem("fl_sem")    # fill DMAs (+48 per tile)
        rbs_sem = sem("rbs_sem")  # readbacks
        va_sem = sem("va_sem")    # tail adds progress
        ar_sem = sem("ar_sem")    # ar_in pushed / out written
        cc_sem = sem("cc_sem")    # collective done

        sb = lambda *a: ctx.enter_context(nc.sbuf_tensor(*a))
        w_rep = sb("w_rep", [NP, C_SH, K], f32)
        patch_sb = sb("patch_sb", [NP, 4, K], f32)
        rel2 = sb("rel2", [NP, N_TILES, C_SH, 2], f32)
        prod = sb("prod", [NP, 2, C_SH, K], f32)
        mx = sb("mx", [NP, 2, C_SH], f32)
        tmp_s = sb("tmp_s", [NP, K], bf16)          # Act Sign scratch
        q = sb("q", [NP, 2, C_SH, K], bf16)  # channel-major: all writes contiguous
        u = sb("u", [NP, 4, K], bf16)
        v = sb("v", [NP, 2, K], bf16)
        Pbuf = sb("Pbuf", [NP, 3, K], bf16)
        zero_t = sb("zero_t", [128, ROW], bf16)
        # [p, slot, canvas, half-slot h, py%4, pc]; h = j*2 + py//4
        bigrb = sb("bigrb", [128, 2, 2, 2 * JS, 4, C_IN], bf16)
        # 16-elem zero guard in front (h slot 0): h index shifted by 1
        acc2 = sb("acc2", [128, 2, 2 * JS + 1, 4, C_IN], bf16)
        accf = sb("accf", [128, 2, JS, C_IN, C_IN], f32)   # [p, slot, gy//4, gy%4, pc]
        psnap = sb("psnap", [NP, K], bf16)
        psnap2 = sb("psnap2", [NP, K], bf16)

        # ---------------- sync engine: patches + fills + tail DMAs ----------
        @block.sync
        def _(sync: bass.BassEngine):
            if with_compute:
                # patch pairs: tiles (2m, 2m+1) -> slots (2m%4, 2m%4+1)
                def load_pair(m):
                    nt = min(2, N_TILES - 2 * m)
                    sync.dma_start(
                        out=patch_sb[:, (2 * m) % 4 : (2 * m) % 4 + nt, :],
                        in_=AP(
                            patches_ext,
                            2 * m * NP * K,
                            [[K, NP], [NP * K, nt], [1, K]],
                        ),
                    ).then_inc(pt_sem, 16)

                # tile-0 patch alone first: it gates Pool's first mult and
                # the shared DMA device serializes transfers
                sync.dma_start(
                    out=patch_sb[:, 0:1, :],
                    in_=AP(patches_ext, 0, [[K, NP], [1, K]]),
                ).then_inc(pt_sem, 16)
                sync.dma_start(
                    out=w_rep[:, 4:, :],
                    in_=AP(w_ext, 4 * K, [[0, NP], [K, C_SH - 4], [1, K]]),
                ).then_inc(zwb_sem, 16)
                sync.dma_start(
                    out=patch_sb[:, 1:2, :],
                    in_=AP(patches_ext, NP * K, [[K, NP], [1, K]]),
                ).then_inc(pt_sem, 16)
                load_pair(1)
                for t in range(N_TILES):
                    # prefetch pair m = t//2 + 2 once tile 2m-3's readers done
                    if t % 2 == 0 and t // 2 + 2 <= (N_TILES - 1) // 2:
                        m = t // 2 + 2
                        sync.wait_ge(pe_sem, 2 * m - 2)
                        load_pair(m)
                    import os as _os3
                    sync.wait_ge(dv_sem, 3 * (t + 1))
                    if with_tail:
                        if t == 0:
                            sync.wait_ge(zc_sem, 16)
                        for il in range(TILE_I):
                            i = TILE_I * t + il
                            a = i % 2
                            sync.dma_start(
                                out=AP(
                                    canv,
                                    a * CANV + 4 * i * ROW,
                                    [[F * C_IN, JS], [ROW, F], [1, F * C_IN]],
                                ),
                                in_=Pbuf[il * JS : (il + 1) * JS, t % 3, :],
                            ).then_inc(fl_sem, 16)
                        if t == 0 and _os3.environ.get("DBG_P0"):
                            sync.dma_start(
                                out=AP(dbg_dram, 0, [[K, NP], [1, K]]),
                                in_=Pbuf[:, 0, :],
                            ).then_inc(fl_sem, 16)
                        if t == 0 and _os3.environ.get("DBG_Q0"):
                            sync.dma_start(
                                out=AP(dbg_f32, 0, [[C_SH, NP], [1, C_SH]]),
                                in_=mx[:, 0, :],
                            ).then_inc(fl_sem, 16)
                            sync.dma_start(
                                out=AP(dbg_dram, 0, [[48, NP], [1, 48]]),
                                in_=qL[:, 0, 0:12, :],
                            ).then_inc(fl_sem, 16)
                            sync.dma_start(
                                out=AP(dbg_dram, NP * 48, [[48, NP], [1, 48]]),
                                in_=qH[:, 0, 0:12, :],
                            ).then_inc(fl_sem, 16)
                        if t == N_TILES - 3:
                            # rows 0-127 (slot 0) final after fills(10)
                            sync.wait_ge(fl_sem, 48 * (N_TILES - 2))
                            sync.dma_start(
                                out=bigrb[:, 0, :, :, :, :],
                                in_=AP(canv, 0, [[ROW, 128], [CANV, 2], [1, ROW]]),
                            ).then_inc(rbs_sem, 16)

            if with_tail:
                # push slot 0 (rows 0-127) once its unslot is done
                sync.wait_ge(va_sem, 2)
                sync.dma_start(
                    out=AP(ar_in, 0, [[WC, 128], [1, WC]]),
                    in_=accf[:, 0, :, :, :],
                ).then_inc(ar_sem, 16)
                sync.wait_ge(va_sem, 4)
                sync.dma_start(
                    out=AP(ar_in, 128 * WC, [[WC, 32], [1, WC]]),
                    in_=accf[0:32, 1, :, :, :],
                ).then_inc(ar_sem, 16)

                sync.wait_ge(cc_sem, 1)
                import os as _os
                if _os.environ.get("DBG_DUMP"):
                    sync.wait_ge(ar_sem, 32)
                else:
                    sync.dma_start(
                        out=AP(out_ext, 0, [[1, RS_SH]]),
                        in_=AP(rs_out, 0, [[1, RS_SH]]),
                    ).then_inc(ar_sem, 16)
                    sync.wait_ge(ar_sem, 48)

        # ---------------- scalar engine (Activation): loads + one-hot -------
        @block.scalar
        def _(scalar: bass.BassScalarEngine):
            # half of w first (other half on the sync queue, in parallel)
            scalar.dma_start(
                out=w_rep[:, 0:4, :],
                in_=AP(w_ext, 0, [[0, NP], [K, 4], [1, K]]),
            ).then_inc(zw_sem, 16)
            scalar.dma_start(
                out=rel2[:, :, :, :], in_=rel_ext[:, :, :, :]
            ).then_inc(rl_sem, 16)
            # preload the activation function table while DMAs stream in
            scalar.activation(
                out=tmp_s[0:1, 0:4],
                in_=tmp_s[0:1, 0:4],
                func=ActF.Sign,
                bias=0.0,
                scale=1.0,
            )
            if with_tail:
                # zero canvas-1 rows {0..3, 156..159} (i odd covers gx 4..155)
                scalar.wait_ge(z_sem, 1)
                scalar.dma_start(
                    out=AP(canv, CANV, [[156 * ROW, 2], [ROW, 4], [1, ROW]]),
                    in_=zero_t[0:8, :],
                ).then_inc(zc_sem, 16)

            if with_compute:
                scalar.wait_ge(rl_sem, 16)
                for t in range(N_TILES):
                    for ci, c in enumerate(range(CP, C_SH)):
                        if ci == 0:
                            # DVE's max lvl2 for tile t done
                            scalar.wait_ge(tr_sem, 5 * t + 1)
                        # s = Sign(-prod + mx) in {0 (argmax), +1}
                        scalar.activation(
                            out=tmp_s[:, :],
                            in_=prod[:, t % 2, c, :],
                            func=ActF.Sign,
                            bias=mx[:, t % 2, c : c + 1],
                            scale=-1.0,
                        ).then_inc(ak_sem, 1)
                        if ci == 0 and t >= 2:
                            # q[t%2] WAR: DVE lvl1(t-2) must have consumed it
                            scalar.wait_ge(dv_sem, 3 * (t - 2) + 1)
                        # q = s*(-rel) + rel  -> rel at argmax, 0 elsewhere
                        scalar.activation(
                            out=q[:, t % 2, c, :],
                            in_=tmp_s[:, :],
                            func=ActF.Identity,
                            bias=rel2[:, t, c, 0:1],
                            scale=rel2[:, t, c, 1:2],
                        ).then_inc(ak_sem, 1)

            if with_tail:
                if with_compute:
                    scalar.wait_ge(fl_sem, 48 * N_TILES)
                else:
                    scalar.wait_ge(zc_sem, 16)
                scalar.dma_start(
                    out=bigrb[0:32, 1, :, :, :, :],
                    in_=AP(canv, 128 * ROW, [[ROW, 32], [CANV, 2], [1, ROW]]),
                ).then_inc(rbs_sem, 16)

        # ---------------- DVE: max8 + STT one-hot c0-2 + add tree -----------
        @block.vector
        def _(vector: bass.BassVectorEngine):
            # canvas-zero source + acc guards: DVE idles at boot anyway
            vector.memset(zero_t[:, :], 0.0)
            vector.memset(acc2[:, :, 0:1, :, :], 0.0)
            vector.memset(acc2[:, 1, :, :, :], 0.0).then_inc(z_sem, 1)

            def tree(tr):
                # q[tr%2] complete: Act(tr) done (own STTs are program-order)
                vector.wait_ge(ak_sem, AI * min(tr + 1, N_TILES - 1))
                vector.tensor_tensor(
                    out=u[:, :, :],
                    in0=q[:, tr % 2, 0:4, :],
                    in1=q[:, tr % 2, 4:8, :],
                    op=Alu.add,
                ).then_inc(dv_sem, 1)
                vector.tensor_tensor(
                    out=v[:, :, :],
                    in0=u[:, 0:2, :],
                    in1=u[:, 2:4, :],
                    op=Alu.add,
                ).then_inc(dv_sem, 1)
                if with_tail and tr >= 3:
                    vector.wait_ge(fl_sem, 48 * (tr - 2))
                vector.tensor_tensor(
                    out=Pbuf[:, tr % 3, :],
                    in0=v[:, 0, :],
                    in1=v[:, 1, :],
                    op=Alu.add,
                ).then_inc(dv_sem, 1)

            if with_compute:
                vector.wait_ge(rl_sem, 16)
                with nc.allow_low_precision("bf16 one-hot relevance pipeline"):
                    for t in range(N_TILES):
                        import os as _os2
                        if _os2.environ.get("DBG_SERIAL") and t >= 2:
                            vector.wait_ge(fl_sem, 48 * (t - 1))
                        # Pool's mult for tile t done
                        vector.wait_ge(pe_sem, t + 1)
                        if t >= 2:
                            # mx[t%2] WAR: Act(t-2) done reading
                            vector.wait_ge(ak_sem, AI * (t - 1))
                        vector.tensor_reduce(
                            out=mx[:, t % 2, :],
                            in_=prod[:, t % 2, :, :],
                            axis=mybir.AxisListType.X,
                            op=Alu.max,
                        ).then_inc(tr_sem, 1)
                        # fused one-hot*rel (all 8 channels on the last
                        # tile so Act retires one tile earlier)
                        for c in range(C_SH if t == N_TILES - 1 else CP):
                            vector.scalar_tensor_tensor(
                                out=q[:, t % 2, c, :],
                                in0=prod[:, t % 2, c, :],
                                scalar=mx[:, t % 2, c : c + 1],
                                in1=rel2[:, t, c, 0]
                                .unsqueeze(1)
                                .to_broadcast([NP, K]),
                                op0=Alu.is_equal,
                                op1=Alu.mult,
                            ).then_inc(tr_sem, 1)
                        if t >= 1:
                            tree(t - 1)
                            import os as _osA
                            if t == 1 and _osA.environ.get("DBG_PSNAP"):
                                vector.tensor_scalar(
                                    out=psnap[:, :],
                                    in0=Pbuf[:, 0, :],
                                    scalar1=1.0,
                                    scalar2=None,
                                    op0=Alu.mult,
                                )
                    tree(N_TILES - 1)
                    import os as _osB
                    if _osB.environ.get("DBG_PSNAP"):
                        vector.tensor_scalar(
                            out=psnap2[:, :],
                            in0=Pbuf[:, (N_TILES - 1) % 3, :],
                            scalar1=1.0,
                            scalar2=None,
                            op0=Alu.mult,
                        )

            if with_tail:
                with nc.allow_low_precision("bf16 canvas sums"):
                    vector.wait_ge(rbs_sem, 16)
                    vector.wait_ge(z_sem, 1)
                    vector.tensor_tensor(
                        out=acc2[:, 0, 1:, :, :],
                        in0=bigrb[:, 0, 0, :, :, :],
                        in1=bigrb[:, 0, 1, :, :, :],
                        op=Alu.add,
                    ).then_inc(va_sem, 1)
                    # un-slot: out[gx, gy, pc] = slot[j(gy), py<4] + slot[j-1, py+4]
                    vector.tensor_tensor(
                        out=accf[:, 0, :, :, :],
                        in0=acc2[:, 0, 1 : 2 * JS + 1 : 2, :, :],
                        in1=acc2[:, 0, 0 : 2 * JS : 2, :, :],
                        op=Alu.add,
                    ).then_inc(va_sem, 1)
                    vector.wait_ge(rbs_sem, 32)
                    vector.tensor_tensor(
                        out=acc2[0:32, 1, 1:, :, :],
                        in0=bigrb[0:32, 1, 0, :, :, :],
                        in1=bigrb[0:32, 1, 1, :, :, :],
                        op=Alu.add,
                    ).then_inc(va_sem, 1)
                    vector.tensor_tensor(
                        out=accf[0:32, 1, :, :, :],
                        in0=acc2[0:32, 1, 1 : 2 * JS + 1 : 2, :, :],
                        in1=acc2[0:32, 1, 0 : 2 * JS : 2, :, :],
                        op=Alu.add,
                    ).then_inc(va_sem, 1)

        # ---------------- Pool: one-hot*rel (c<CP) + lvl1 + collective ------

        @block.gpsimd
        def _(gpsimd: bass.BassGpSimd):
            if with_compute:
                gpsimd.wait_ge(zw_sem, 16)
                gpsimd.wait_ge(zwb_sem, 16)
                with nc.allow_low_precision("bf16 one-hot relevance pipeline"):
                    for t in range(N_TILES):
                        gpsimd.wait_ge(pt_sem, 16 * (t + 1) if t < 2 else 16 * (t // 2 + 2))
                        if t >= 2:
                            # prod[t%2] WAR: Act(t-2) + DVE(t-2) done reading
                            gpsimd.wait_ge(ak_sem, AI * (t - 1))
                            gpsimd.wait_ge(tr_sem, 5 * (t - 1))
                        gpsimd.tensor_tensor(
                            out=prod[:, t % 2, :, :],
                            in0=patch_sb[:, t % 4, :]
                            .unsqueeze(1)
                            .to_broadcast([NP, C_SH, K]),
                            in1=w_rep[:, :, :],
                            op=Alu.mult,
                        ).then_inc(pe_sem, 1)

            if with_tail:
                import os as _os
                if _os.environ.get("DBG_DUMP"):
                    gpsimd.wait_ge(fl_sem, 48 * N_TILES)
                    _r0 = int(_os.environ.get("DBG_ROW", "0"))
                    _cv = int(_os.environ.get("DBG_CANV", "0"))
                    gpsimd.dma_start(
                        out=AP(out_ext, 0, [[1, 12800]]),
                        in_=AP(canv, _cv * CANV + _r0 * ROW, [[1, 12800]]),
                    ).then_inc(ar_sem, 16)
                gpsimd.wait_ge(ar_sem, 32)
                gpsimd.collective_compute(
                    "ReduceScatter",
                    mybir.AluOpType.add,
                    replica_groups=[list(range(N_CORES))],
                    ins=[ar_in[:]],
                    outs=[rs_out[:]],
                ).then_inc(cc_sem, 1)

    return nc


_NC = None


def _get_nc():
    global _NC
    if _NC is None:
        _NC = _build_nc()
    return _NC


LAST_RESULT = None


def kernel(inputs, layer_output, layer_weights, stride=4, filter_size=8, **_kw):
    assert int(stride) == S and int(filter_size) == F
    rel = np.asarray(inputs, dtype=np.float32)[0]          # [39,39,64]
    x = np.ascontiguousarray(np.asarray(layer_output, dtype=np.float32)[0])
    w = np.asarray(layer_weights, dtype=np.float32)        # [8,8,4,64]

    import ml_dtypes

    # host-side im2col in (t, il*40+j, k) layout, natural j order, j=39 padded
    idx_r = (S * np.arange(H_OUT))[:, None] + np.arange(F)[None, :]
    idx_c = (S * np.arange(W_OUT))[:, None] + np.arange(F)[None, :]
    pat = x[idx_r][:, :, idx_c, :]                    # [i, px, j, py, pc]
    pat = pat.transpose(0, 2, 1, 3, 4).reshape(H_OUT, W_OUT, K)
    pat40 = np.concatenate([pat, pat[:, -1:, :]], axis=1)   # pad j=39 (finite)
    patches = np.ascontiguousarray(pat40.reshape(N_TILES, NP, K))

    from concourse.bass_utils import run_bass_kernel_spmd

    nc = _get_nc()
    in_maps = []
    for r in range(N_CORES):
        cs = slice(C_SH * r, C_SH * (r + 1))
        w_t = np.ascontiguousarray(
            w[:, :, :, cs].transpose(3, 0, 1, 2).reshape(C_SH, K)
        )
        rel_r = rel[:, :, cs]                              # [39, 39, 8]
        rel_p = np.zeros((H_OUT, JS, C_SH, 2), dtype=np.float32)
        rel_p[:, :W_OUT, :, 0] = rel_r
        rel_p[:, :W_OUT, :, 1] = -rel_r
        rel_p = np.ascontiguousarray(
            rel_p.reshape(N_TILES, NP, C_SH, 2).transpose(1, 0, 2, 3)
        )
        in_maps.append({"patches": patches, "w": w_t, "rel": rel_p})

    import os

    trace = bool(int(os.environ.get("KERNEL_TRACE", "0")))
    res = run_bass_kernel_spmd(nc, in_maps, list(range(N_CORES)), trace=trace)
    global LAST_RESULT
    LAST_RESULT = res
    slices = [np.asarray(res.results[r]["out"]) for r in range(N_CORES)]
    out = np.concatenate(slices, axis=0).reshape(1, H_IN, W_IN, C_IN)
    return out.astype(np.float32)
